# revision 4
# baseline (speedup 1.0000x reference)
"""Trainium2 Bass kernel for EnhancedPortfolioGAT (2-layer GAT + BN + MLP head).

Strategy (graph/data parallel over 8 NeuronCores):
 - Nodes are sharded row-wise: core c owns nodes [c*6250, (c+1)*6250).
 - Edges (plus self-loops) are routed to the core owning their destination,
   sorted by destination, grouped into 128-node destination tiles and
   128-edge chunks.
 - GAT segment-softmax aggregation per chunk: indirect-DMA gather of source
   rows from an augmented node table G=[h | s_src], per-edge attention
   weight ex=exp(leakyrelu(s_src+s_dst)), and a one-hot matmul that
   scatter-adds [ex | ex*h] into a PSUM accumulator per destination tile
   (denominator and numerator in one matmul).
 - All BatchNorm / bias affine transforms are folded into the weight
   matrices on the host. The per-layer feature transform is computed once
   per 128-node tile as a single matmul against [W | W@A] so the attention
   projections (s_src, s_dst) come out of the same PSUM.
 - Two SPMD launches: launch 1 = node phase A (replicated) + conv1 edge
   phase + conv2 node phase; host gathers per-core G2 shards (halo
   exchange); launch 2 = conv2 edge phase + MLP head.
"""

import math

import numpy as np
import ml_dtypes

import concourse.bass as bass
import concourse.tile as tile
from concourse import bacc, mybir
from concourse.bass_utils import run_bass_kernel_spmd

BF16 = ml_dtypes.bfloat16
P = 128

# Problem configuration (hardcoded per the harness contract).
N = 50000
E = 800000
NCORES = 8
HEADS = 8
HID = 32
DIN = 64
WDIM = HEADS * HID          # 256
GW = WDIM + HEADS           # 264: [h (256) | s_src (8)]
AW = WDIM + 2 * HEADS       # 272: [h (256) | s_src (8) | s_dst (8)]
KA = DIN + 1                # 65: x plus ones column
NPC = N // NCORES           # 6250 nodes per core
NEG_SLOPE = 0.2
BN_EPS = 1e-5

F32 = mybir.dt.float32
BF = mybir.dt.bfloat16
I32 = mybir.dt.int32

_PROG_CACHE = {}

# Runtime knobs for experiments (test.py may override).
TRACE = False
TRACE_KW = {}


def _ceil(a, b):
    return -(-a // b)


# ---------------------------------------------------------------------------
# Host-side parameter folding
# ---------------------------------------------------------------------------

def _fold(inp):
    f = lambda k: inp[k].astype(np.float64)

    def bn_fold(pre):
        q = f(pre + "_g") / np.sqrt(f(pre + "_v") + BN_EPS)
        r = f(pre + "_b") - f(pre + "_m") * q
        return q, r

    def a_mat(a_src, a_dst):
        # A[h*HID+d, j]: j<HEADS -> a_src[h,d] if j==h ; j>=HEADS -> a_dst
        A = np.zeros((WDIM, 2 * HEADS))
        for h in range(HEADS):
            A[h * HID:(h + 1) * HID, h] = a_src[h]
            A[h * HID:(h + 1) * HID, HEADS + h] = a_dst[h]
        return A

    out = {}
    # conv1: G1 = bn1(x) @ [W1 | W1@A1]  (+ const row via ones column of x)
    q1, r1 = bn_fold("bn1")
    W1f = q1[:, None] * f("W1")
    d1 = r1 @ f("W1")
    A1 = a_mat(f("a1_src"), f("a1_dst"))
    W1ext = np.concatenate([W1f, W1f @ A1], 1)            # [64, 272]
    d1ext = np.concatenate([d1, d1 @ A1])                 # [272]
    out["W1aug"] = np.vstack([W1ext, d1ext]).astype(BF16)  # [65, 272]

    # conv2 node phase: G2 = bn2(e1) @ [W2 | W2@A2] = e1 @ W2ext + d2ext
    q2, r2 = bn_fold("bn2")
    W2f = q2[:, None] * f("W2")
    d2 = r2 @ f("W2")
    A2 = a_mat(f("a2_src"), f("a2_dst"))
    W2ext = np.concatenate([W2f, W2f @ A2], 1)            # [256, 272]
    d2ext = np.concatenate([d2, d2 @ A2])                 # [272]
    out["W2a0"] = W2ext[0:128].astype(BF16)
    out["W2a1"] = W2ext[128:256].astype(BF16)
    out["W2d"] = d2ext[None, :].astype(BF16)              # [1, 272]

    # head: t = relu(e2 @ P1a + x_aug @ P1baug); y = t @ p2 + p2_b
    q3, r3 = bn_fold("bn3")
    P1a = q3[:, None] * f("p1_W")                         # [256, 32]
    P1b = f("skip_W") @ f("p1_W")                         # [64, 32]
    cP1 = r3 @ f("p1_W") + f("p1_b") + f("skip_b") @ f("p1_W")  # [32]
    out["P1a0"] = P1a[0:128].astype(BF16)
    out["P1a1"] = P1a[128:256].astype(BF16)
    out["P1baug"] = np.vstack([P1b, cP1]).astype(BF16)    # [65, 32]
    out["p2"] = f("p2_W").astype(BF16)                    # [32, 1]
    out["p2brep"] = np.full((P, 1), float(inp["p2_b"][0]), np.float32)

    out["b1rep"] = np.broadcast_to(
        inp["b1"].astype(np.float32), (P, WDIM)).copy()
    out["b2rep"] = np.broadcast_to(
        inp["b2"].astype(np.float32), (P, WDIM)).copy()

    # x augmented with ones column, transposed, tiled: [NTA, 65, 128]
    x = inp["x"].astype(np.float32)
    nta = _ceil(N, P)
    xa = np.zeros((nta * P, KA), np.float32)
    xa[:N, :DIN] = x
    xa[:N, DIN] = 1.0
    out["xaT"] = np.ascontiguousarray(
        xa.reshape(nta, P, KA).transpose(0, 2, 1)).astype(BF16)

    # per-core own x tiles for the head
    tiles_own = _ceil(NPC, P)
    xaTown = []
    for c in range(NCORES):
        xo = np.zeros((tiles_own * P, KA), np.float32)
        xo[:NPC, :DIN] = x[c * NPC:(c + 1) * NPC]
        xo[:NPC, DIN] = 1.0
        xaTown.append(np.ascontiguousarray(
            xo.reshape(tiles_own, P, KA).transpose(0, 2, 1)).astype(BF16))
    out["xaTown"] = xaTown

    out["iota"] = np.broadcast_to(
        np.arange(P, dtype=np.float32), (P, P)).astype(BF16).copy()
    out["ident"] = np.eye(P, dtype=np.float32).astype(BF16)
    out["ones"] = np.ones((1, P), np.float32).astype(BF16)
    return out


# ---------------------------------------------------------------------------
# Host-side edge planning
# ---------------------------------------------------------------------------

def _plan_edges(edge_index):
    src = edge_index[0].astype(np.int64)
    dst = edge_index[1].astype(np.int64)
    loops = np.arange(N, dtype=np.int64)
    src = np.concatenate([src, loops])
    dst = np.concatenate([dst, loops])

    tiles_own = _ceil(NPC, P)
    core_of = dst // NPC

    per_core = []
    counts_all = np.zeros((NCORES, tiles_own), np.int64)
    for c in range(NCORES):
        m = core_of == c
        s = src[m]
        d = dst[m] - c * NPC
        order = np.argsort(d, kind="stable")
        s, d = s[order], d[order]
        t = d // P
        counts_all[c] = np.bincount(t, minlength=tiles_own)
        per_core.append((s, d, t))

    # per-tile chunk count = max over cores (program structure is shared)
    C = np.maximum(_ceil_arr(counts_all.max(0), P), 1)
    coloff = np.concatenate([[0], np.cumsum(C)])
    totc = int(coloff[-1])

    srcidx = np.zeros((NCORES, P, totc), np.int32)
    dstidx = np.zeros((NCORES, P, totc), np.int32)
    dstloc = np.full((NCORES, P, totc), 255.0, np.float32)
    for c in range(NCORES):
        s, d, t = per_core[c]
        seg_start = np.concatenate([[0], np.cumsum(counts_all[c])])[:-1]
        r = np.arange(len(d)) - seg_start[t]
        col = coloff[t] + r // P
        p = r % P
        srcidx[c, p, col] = s
        dstidx[c, p, col] = d + c * NPC
        dstloc[c, p, col] = d % P
    return {
        "C": tuple(int(v) for v in C),
        "totc": totc,
        "srcidx": srcidx,
        "dstidx": dstidx,
        "dstloc": dstloc.astype(BF16),
        "tiles_own": tiles_own,
    }


def _ceil_arr(a, b):
    return -(-a // b)


# ---------------------------------------------------------------------------
# Device program builders
# ---------------------------------------------------------------------------

def _emit_edge_phase(nc, tc, pools, C, g_ap, sd_ap, src_sb, dst_sb, loc_sb,
                     iota_sb, tile_epilogue):
    """Edge aggregation over destination tiles; per tile calls
    tile_epilogue(t, psumB) with the accumulated [P, GW] PSUM
    (cols 0:8 = sum(ex) per head, cols 8:264 = sum(ex*h))."""
    sbp, psp = pools
    col = 0
    for t in range(len(C)):
        psumB = psp.tile([P, GW], F32, space="PSUM", tag="psumB")
        for ch in range(C[t]):
            g = sbp.tile([P, GW], BF, tag="gather")
            nc.gpsimd.indirect_dma_start(
                out=g[:], out_offset=None, in_=g_ap[:],
                in_offset=bass.IndirectOffsetOnAxis(
                    ap=src_sb[:, col:col + 1], axis=0),
            )
            sd = sbp.tile([P, HEADS], BF, tag="sdg")
            nc.gpsimd.indirect_dma_start(
                out=sd[:], out_offset=None, in_=sd_ap[:],
                in_offset=bass.IndirectOffsetOnAxis(
                    ap=dst_sb[:, col:col + 1], axis=0),
            )
            score = sbp.tile([P, HEADS], F32, tag="score")
            nc.vector.tensor_tensor(
                out=score[:], in0=g[:, WDIM:GW], in1=sd[:],
                op=mybir.AluOpType.add)
            score2 = sbp.tile([P, HEADS], F32, tag="score2")
            nc.scalar.activation(
                score2[:], score[:], mybir.ActivationFunctionType.Lrelu,
                alpha=NEG_SLOPE)
            rhs = sbp.tile([P, GW], BF, tag="rhs")
            nc.scalar.activation(
                rhs[:, 0:HEADS], score2[:], mybir.ActivationFunctionType.Exp)
            nc.vector.tensor_tensor(
                out=rhs[:, HEADS:GW].rearrange("p (h d) -> p h d", h=HEADS),
                in0=g[:, 0:WDIM].rearrange("p (h d) -> p h d", h=HEADS),
                in1=rhs[:, 0:HEADS].unsqueeze(-1).to_broadcast(
                    [P, HEADS, HID]),
                op=mybir.AluOpType.mult)
            oh = sbp.tile([P, P], BF, tag="onehot")
            nc.vector.tensor_tensor(
                out=oh[:], in0=loc_sb[:, col:col + 1].to_broadcast([P, P]),
                in1=iota_sb[:], op=mybir.AluOpType.is_equal)
            nc.tensor.matmul(
                out=psumB[:], lhsT=oh[:], rhs=rhs[:],
                start=(ch == 0), stop=(ch == C[t] - 1))
            col += 1
        tile_epilogue(t, psumB)


def _emit_softmax_elu(nc, sbp, psumB, brep_sb):
    """From edge-phase PSUM to e = elu(num/den + b): returns [P, WDIM] f32."""
    den = sbp.tile([P, HEADS], F32, tag="den")
    nc.vector.tensor_scalar(
        out=den[:], in0=psumB[:, 0:HEADS], scalar1=1e-30, scalar2=None,
        op0=mybir.AluOpType.max)
    recip = sbp.tile([P, HEADS], F32, tag="recip")
    nc.vector.reciprocal(recip[:], den[:])
    o = sbp.tile([P, WDIM], F32, tag="agg")
    nc.vector.tensor_tensor(
        out=o[:].rearrange("p (h d) -> p h d", h=HEADS),
        in0=psumB[:, HEADS:GW].rearrange("p (h d) -> p h d", h=HEADS),
        in1=recip[:].unsqueeze(-1).to_broadcast([P, HEADS, HID]),
        op=mybir.AluOpType.mult)
    ob = sbp.tile([P, WDIM], F32, tag="aggb")
    nc.vector.tensor_tensor(
        out=ob[:], in0=o[:], in1=brep_sb[:], op=mybir.AluOpType.add)
    neg = sbp.tile([P, WDIM], F32, tag="neg")
    nc.vector.tensor_scalar(
        out=neg[:], in0=ob[:], scalar1=0.0, scalar2=None,
        op0=mybir.AluOpType.min)
    en = sbp.tile([P, WDIM], F32, tag="en")
    nc.scalar.activation(en[:], neg[:], mybir.ActivationFunctionType.Exp)
    pm1 = sbp.tile([P, WDIM], F32, tag="pm1")
    nc.vector.tensor_scalar(
        out=pm1[:], in0=ob[:], scalar1=0.0, scalar2=-1.0,
        op0=mybir.AluOpType.max, op1=mybir.AluOpType.add)
    e = sbp.tile([P, WDIM], F32, tag="e")
    nc.vector.tensor_tensor(
        out=e[:], in0=en[:], in1=pm1[:], op=mybir.AluOpType.add)
    return e


def _emit_transpose_halves(nc, sbp, psp, e, ident_sb):
    """e [P, WDIM] f32 -> (e_bf, [eT0, eT1]) with eTi [P, P] bf16 = e.T halves."""
    eb = sbp.tile([P, WDIM], BF, tag="eb")
    nc.vector.tensor_copy(out=eb[:], in_=e[:])
    eTs = []
    for half in range(2):
        pst = psp.tile([P, P], BF, space="PSUM", tag="psT")
        nc.tensor.transpose(
            out=pst[:], in_=eb[:, half * P:(half + 1) * P], identity=ident_sb[:])
        eT = sbp.tile([P, P], BF, tag=f"eT{half}")
        nc.vector.tensor_copy(out=eT[:], in_=pst[:])
        eTs.append(eT)
    return eTs


def _build_launch1(C, tiles_own, totc, nta):
    nc = bacc.Bacc("TRN2", target_bir_lowering=False, debug=False,
                   enable_asserts=False, num_devices=NCORES)
    dt = nc.dram_tensor
    xaT = dt("xaT", [nta, KA, P], BF, kind="ExternalInput").ap()
    W1aug = dt("W1aug", [KA, AW], BF, kind="ExternalInput").ap()
    W2a0 = dt("W2a0", [P, AW], BF, kind="ExternalInput").ap()
    W2a1 = dt("W2a1", [P, AW], BF, kind="ExternalInput").ap()
    W2d = dt("W2d", [1, AW], BF, kind="ExternalInput").ap()
    iota = dt("iota", [P, P], BF, kind="ExternalInput").ap()
    ident = dt("ident", [P, P], BF, kind="ExternalInput").ap()
    ones = dt("ones", [1, P], BF, kind="ExternalInput").ap()
    b1rep = dt("b1rep", [P, WDIM], F32, kind="ExternalInput").ap()
    srcidx = dt("srcidx", [P, totc], I32, kind="ExternalInput").ap()
    dstidx = dt("dstidx", [P, totc], I32, kind="ExternalInput").ap()
    dstloc = dt("dstloc", [P, totc], BF, kind="ExternalInput").ap()
    g2own = dt("g2own", [NPC, GW], BF, kind="ExternalOutput").ap()
    sd2own = dt("sd2own", [NPC, HEADS], BF, kind="ExternalOutput").ap()
    G1 = dt("G1", [nta * P, GW], BF).ap()
    SD1 = dt("SD1", [nta * P, HEADS], BF).ap()

    with tile.TileContext(nc) as tc:
        with (
            tc.tile_pool(name="consts", bufs=1) as cst,
            tc.tile_pool(name="sbuf", bufs=6) as sbp,
            tc.tile_pool(name="sbuf2", bufs=2) as sb2,
            tc.tile_pool(name="psA", bufs=2, space="PSUM") as psA,
            tc.tile_pool(name="psB", bufs=2, space="PSUM") as psB,
            tc.tile_pool(name="psT", bufs=2, space="PSUM") as psT,
        ):
            def cload(ap, shape, dtype):
                t = cst.tile(shape, dtype, tag=ap.tensor.name)
                nc.sync.dma_start(t[:], ap[:])
                return t

            W1aug_sb = cload(W1aug, [KA, AW], BF)
            W2a0_sb = cload(W2a0, [P, AW], BF)
            W2a1_sb = cload(W2a1, [P, AW], BF)
            W2d_sb = cload(W2d, [1, AW], BF)
            iota_sb = cload(iota, [P, P], BF)
            ident_sb = cload(ident, [P, P], BF)
            ones_sb = cload(ones, [1, P], BF)
            b1rep_sb = cload(b1rep, [P, WDIM], F32)
            src_sb = cload(srcidx, [P, totc], I32)
            dst_sb = cload(dstidx, [P, totc], I32)
            loc_sb = cload(dstloc, [P, totc], BF)

            # ---- phase A: full-graph node transform (replicated) ----
            for t in range(nta):
                xt = sbp.tile([KA, P], BF, tag="xt")
                nc.sync.dma_start(xt[:], xaT[t])
                psa = psA.tile([P, AW], F32, space="PSUM", tag="psumA")
                nc.tensor.matmul(out=psa[:], lhsT=xt[:], rhs=W1aug_sb[:],
                                 start=True, stop=True)
                g1 = sbp.tile([P, GW], BF, tag="g1")
                nc.vector.tensor_copy(out=g1[:], in_=psa[:, 0:GW])
                nc.sync.dma_start(G1[t * P:(t + 1) * P, :], g1[:])
                sd1 = sbp.tile([P, HEADS], BF, tag="sd1")
                nc.vector.tensor_copy(out=sd1[:], in_=psa[:, GW:AW])
                nc.sync.dma_start(SD1[t * P:(t + 1) * P, :], sd1[:])

            tc.strict_bb_all_engine_barrier()

            # ---- phase B1 + conv2 node phase ----
            def epilogue(t, psumB):
                rows = min(NPC - t * P, P)
                e1 = _emit_softmax_elu(nc, sb2, psumB, b1rep_sb)
                eTs = _emit_transpose_halves(nc, sb2, psT, e1, ident_sb)
                psa2 = psA.tile([P, AW], F32, space="PSUM", tag="psumA")
                nc.tensor.matmul(out=psa2[:], lhsT=ones_sb[:], rhs=W2d_sb[:],
                                 start=True, stop=False)
                nc.tensor.matmul(out=psa2[:], lhsT=eTs[0][:], rhs=W2a0_sb[:],
                                 start=False, stop=False)
                nc.tensor.matmul(out=psa2[:], lhsT=eTs[1][:], rhs=W2a1_sb[:],
                                 start=False, stop=True)
                g2 = sb2.tile([P, GW], BF, tag="g2")
                nc.vector.tensor_copy(out=g2[:], in_=psa2[:, 0:GW])
                nc.sync.dma_start(
                    g2own[t * P:t * P + rows, :], g2[0:rows, :])
                sd2 = sb2.tile([P, HEADS], BF, tag="sd2")
                nc.vector.tensor_copy(out=sd2[:], in_=psa2[:, GW:AW])
                nc.sync.dma_start(
                    sd2own[t * P:t * P + rows, :], sd2[0:rows, :])

            _emit_edge_phase(nc, tc, (sbp, psB), C, G1, SD1,
                             src_sb, dst_sb, loc_sb, iota_sb, epilogue)
    nc.compile()
    return nc


def _build_launch2(C, tiles_own, totc):
    nc = bacc.Bacc("TRN2", target_bir_lowering=False, debug=False,
                   enable_asserts=False, num_devices=NCORES)
    dt = nc.dram_tensor
    G2 = dt("G2", [N, GW], BF, kind="ExternalInput").ap()
    SD2 = dt("SD2", [N, HEADS], BF, kind="ExternalInput").ap()
    iota = dt("iota", [P, P], BF, kind="ExternalInput").ap()
    ident = dt("ident", [P, P], BF, kind="ExternalInput").ap()
    b2rep = dt("b2rep", [P, WDIM], F32, kind="ExternalInput").ap()
    P1a0 = dt("P1a0", [P, HID], BF, kind="ExternalInput").ap()
    P1a1 = dt("P1a1", [P, HID], BF, kind="ExternalInput").ap()
    P1baug = dt("P1baug", [KA, HID], BF, kind="ExternalInput").ap()
    p2 = dt("p2", [HID, 1], BF, kind="ExternalInput").ap()
    p2brep = dt("p2brep", [P, 1], F32, kind="ExternalInput").ap()
    xaTown = dt("xaTown", [tiles_own, KA, P], BF, kind="ExternalInput").ap()
    srcidx = dt("srcidx", [P, totc], I32, kind="ExternalInput").ap()
    dstidx = dt("dstidx", [P, totc], I32, kind="ExternalInput").ap()
    dstloc = dt("dstloc", [P, totc], BF, kind="ExternalInput").ap()
    y = dt("y", [NPC, 1], F32, kind="ExternalOutput").ap()

    with tile.TileContext(nc) as tc:
        with (
            tc.tile_pool(name="consts", bufs=1) as cst,
            tc.tile_pool(name="sbuf", bufs=6) as sbp,
            tc.tile_pool(name="sbuf2", bufs=2) as sb2,
            tc.tile_pool(name="psB", bufs=2, space="PSUM") as psB,
            tc.tile_pool(name="psT", bufs=2, space="PSUM") as psT,
            tc.tile_pool(name="psC", bufs=1, space="PSUM") as psC,
            tc.tile_pool(name="psT2", bufs=1, space="PSUM") as psT2,
            tc.tile_pool(name="psY", bufs=1, space="PSUM") as psY,
        ):
            def cload(ap, shape, dtype):
                t = cst.tile(shape, dtype, tag=ap.tensor.name)
                nc.sync.dma_start(t[:], ap[:])
                return t

            iota_sb = cload(iota, [P, P], BF)
            ident_sb = cload(ident, [P, P], BF)
            b2rep_sb = cload(b2rep, [P, WDIM], F32)
            P1a0_sb = cload(P1a0, [P, HID], BF)
            P1a1_sb = cload(P1a1, [P, HID], BF)
            P1baug_sb = cload(P1baug, [KA, HID], BF)
            p2_sb = cload(p2, [HID, 1], BF)
            p2b_sb = cload(p2brep, [P, 1], F32)
            src_sb = cload(srcidx, [P, totc], I32)
            dst_sb = cload(dstidx, [P, totc], I32)
            loc_sb = cload(dstloc, [P, totc], BF)

            def epilogue(t, psumB):
                rows = min(NPC - t * P, P)
                e2 = _emit_softmax_elu(nc, sb2, psumB, b2rep_sb)
                eTs = _emit_transpose_halves(nc, sb2, psT, e2, ident_sb)
                xt = sb2.tile([KA, P], BF, tag="xt")
                nc.sync.dma_start(xt[:], xaTown[t])
                psc = psC.tile([P, HID], F32, space="PSUM", tag="psumC")
                nc.tensor.matmul(out=psc[:], lhsT=eTs[0][:], rhs=P1a0_sb[:],
                                 start=True, stop=False)
                nc.tensor.matmul(out=psc[:], lhsT=eTs[1][:], rhs=P1a1_sb[:],
                                 start=False, stop=False)
                nc.tensor.matmul(out=psc[:], lhsT=xt[:], rhs=P1baug_sb[:],
                                 start=False, stop=True)
                tt = sb2.tile([P, HID], BF, tag="tt")
                nc.scalar.activation(tt[:], psc[:],
                                     mybir.ActivationFunctionType.Relu)
                pst2 = psT2.tile([HID, P], BF, space="PSUM", tag="psumT2")
                nc.tensor.transpose(out=pst2[:], in_=tt[:],
                                    identity=ident_sb[:])
                ttT = sb2.tile([HID, P], BF, tag="ttT")
                nc.vector.tensor_copy(out=ttT[:], in_=pst2[:])
                psy = psY.tile([P, 1], F32, space="PSUM", tag="psumY")
                nc.tensor.matmul(out=psy[:], lhsT=ttT[:], rhs=p2_sb[:],
                                 start=True, stop=True)
                ysb = sb2.tile([P, 1], F32, tag="ysb")
                nc.scalar.activation(ysb[:], psy[:],
                                     mybir.ActivationFunctionType.Identity,
                                     bias=p2b_sb[:])
                nc.sync.dma_start(y[t * P:t * P + rows, :], ysb[0:rows, :])

            _emit_edge_phase(nc, tc, (sbp, psB), C, G2, SD2,
                             src_sb, dst_sb, loc_sb, iota_sb, epilogue)
    nc.compile()
    return nc


# ---------------------------------------------------------------------------
# Entry point
# ---------------------------------------------------------------------------

def _get_programs(C, tiles_own, totc, nta):
    key = (C, tiles_own, totc, nta)
    if key not in _PROG_CACHE:
        _PROG_CACHE[key] = (
            _build_launch1(C, tiles_own, totc, nta),
            _build_launch2(C, tiles_own, totc),
        )
    return _PROG_CACHE[key]


def kernel(**inputs):
    cfg = _fold(inputs)
    plan = _plan_edges(inputs["edge_index"])
    C, totc, tiles_own = plan["C"], plan["totc"], plan["tiles_own"]
    nta = _ceil(N, P)
    nc1, nc2 = _get_programs(C, tiles_own, totc, nta)

    shared1 = {k: cfg[k] for k in ["xaT", "W1aug", "W2a0", "W2a1", "W2d",
                                   "iota", "ident", "ones", "b1rep"]}
    in_maps1 = []
    for c in range(NCORES):
        m = dict(shared1)
        m["srcidx"] = plan["srcidx"][c]
        m["dstidx"] = plan["dstidx"][c]
        m["dstloc"] = plan["dstloc"][c]
        in_maps1.append(m)
    res1 = run_bass_kernel_spmd(nc1, in_maps1, list(range(NCORES)),
                                trace=TRACE, **TRACE_KW)
    G2 = np.concatenate([res1.results[c]["g2own"] for c in range(NCORES)], 0)
    SD2 = np.concatenate([res1.results[c]["sd2own"] for c in range(NCORES)], 0)

    shared2 = {k: cfg[k] for k in ["iota", "ident", "b2rep", "P1a0", "P1a1",
                                   "P1baug", "p2", "p2brep"]}
    shared2["G2"] = G2
    shared2["SD2"] = SD2
    in_maps2 = []
    for c in range(NCORES):
        m = dict(shared2)
        m["xaTown"] = cfg["xaTown"][c]
        m["srcidx"] = plan["srcidx"][c]
        m["dstidx"] = plan["dstidx"][c]
        m["dstloc"] = plan["dstloc"][c]
        in_maps2.append(m)
    res2 = run_bass_kernel_spmd(nc2, in_maps2, list(range(NCORES)),
                                trace=TRACE, **TRACE_KW)
    y = np.concatenate([res2.results[c]["y"] for c in range(NCORES)], 0)
    kernel.last_exec_ns = (
        (res1.exec_time_ns or 0) + (res2.exec_time_ns or 0)) or None
    kernel.last_results = (res1, res2)
    return y.astype(np.float32)


# revision 6
# speedup vs baseline: 2.9218x; 2.9218x over previous
"""Trainium2 Bass kernel for EnhancedPortfolioGAT (2-layer GAT + BN + MLP head).

Strategy (graph/data parallel over 8 NeuronCores):
 - Nodes sharded row-wise in 6272-node (49-tile) windows; core c owns
   global nodes [c*6272, min((c+1)*6272, 50000)). Each core works in a
   node numbering ROTATED by c*6272 so its own nodes are positions
   0..6271 -- all core-dependence lives in host-prepared inputs and the
   SPMD program is identical across cores.
 - Edges (plus self-loops) are routed to the core owning their
   destination and grouped into 128-node destination tiles and 128-edge
   chunks.
 - Source rows are fetched per destination tile with dma_gather (int16
   indices; table split at row 32768 into LO/HI halves; <=1024 rows per
   op; round-robined over 4 SWDGE queues). Rows are padded to 768 bytes
   (dma_gather needs 256B multiples).
 - Segment softmax via one-hot matmul: per chunk a [edge x node] one-hot
   built with is_equal scatter-adds [ex*h | ex] into a PSUM accumulator
   (numerator and denominator from one matmul). s_dst is expanded
   edge-wise with a transposed-one-hot matmul against a [128, 8] node
   table instead of a gather.
 - BatchNorm/bias affines folded into weights host-side; each layer's
   node transform emits [h | s_src | s_dst] from a single matmul against
   [W | W@A].
 - Two SPMD launches: launch 1 = full-graph node transform (replicated)
   + conv1 edge phase + conv2 node transform; the host concatenates the
   per-core G2 shards (halo exchange); launch 2 = conv2 edge phase + MLP
   head. Host rolls/gathers are pure data marshalling.
"""

import numpy as np
import ml_dtypes

import concourse.bass as bass
import concourse.tile as tile
from concourse import bacc, mybir
from concourse.bass_utils import run_bass_kernel_spmd

BF16 = ml_dtypes.bfloat16
P = 128

N = 50000
NCORES = 8
HEADS = 8
HID = 32
DIN = 64
WDIM = HEADS * HID          # 256
GW = WDIM + HEADS           # 264 used cols: [msg/h (256) | s_src->ex (8)]
GP = 384                    # padded gather row width (768B)
AW = WDIM + 2 * HEADS       # 272: [h | s_src | s_dst]
KA = DIN + 1                # x plus ones column
NPC = 6272                  # own-window size (49 tiles); last core partial
TILES_OWN = NPC // P        # 49
NEG_SLOPE = 0.2
BN_EPS = 1e-5
SPLIT = 32768
GMAX = 8                    # chunks per dma_gather op (1024 rows)
ABATCH = 4                  # phase-A tiles per iteration
PAD_N = 50176               # ceil(50000/512)*512 : divisible by ABATCH*P

F32 = mybir.dt.float32
BF = mybir.dt.bfloat16
I16 = mybir.dt.int16

_PROG_CACHE = {}

TRACE = False
TRACE_KW = {}


def _ceil(a, b):
    return -(-a // b)


def _npc_real(c):
    return min(NPC, N - c * NPC)


# ---------------------------------------------------------------------------
# Host-side parameter folding
# ---------------------------------------------------------------------------

def _fold(inp):
    f = lambda k: inp[k].astype(np.float64)

    def bn_fold(pre):
        q = f(pre + "_g") / np.sqrt(f(pre + "_v") + BN_EPS)
        r = f(pre + "_b") - f(pre + "_m") * q
        return q, r

    def a_mat(a_src, a_dst):
        A = np.zeros((WDIM, 2 * HEADS))
        for h in range(HEADS):
            A[h * HID:(h + 1) * HID, h] = a_src[h]
            A[h * HID:(h + 1) * HID, HEADS + h] = a_dst[h]
        return A

    out = {}
    q1, r1 = bn_fold("bn1")
    W1f = q1[:, None] * f("W1")
    d1 = r1 @ f("W1")
    A1 = a_mat(f("a1_src"), f("a1_dst"))
    W1ext = np.concatenate([W1f, W1f @ A1], 1)
    d1ext = np.concatenate([d1, d1 @ A1])
    out["W1aug"] = np.vstack([W1ext, d1ext]).astype(BF16)   # [65, 272]

    q2, r2 = bn_fold("bn2")
    W2f = q2[:, None] * f("W2")
    d2 = r2 @ f("W2")
    A2 = a_mat(f("a2_src"), f("a2_dst"))
    W2ext = np.concatenate([W2f, W2f @ A2], 1)
    d2ext = np.concatenate([d2, d2 @ A2])
    out["W2a0"] = W2ext[0:128].astype(BF16)
    out["W2a1"] = W2ext[128:256].astype(BF16)
    out["W2d"] = d2ext[None, :].astype(BF16)

    q3, r3 = bn_fold("bn3")
    P1a = q3[:, None] * f("p1_W")
    P1b = f("skip_W") @ f("p1_W")
    cP1 = r3 @ f("p1_W") + f("p1_b") + f("skip_b") @ f("p1_W")
    out["P1a0"] = P1a[0:128].astype(BF16)
    out["P1a1"] = P1a[128:256].astype(BF16)
    out["P1baug"] = np.vstack([P1b, cP1]).astype(BF16)
    out["p2"] = f("p2_W").astype(BF16)
    out["p2brep"] = np.full((P, 1), float(inp["p2_b"][0]), np.float32)

    out["b1rep"] = np.broadcast_to(
        inp["b1"].astype(np.float32), (P, WDIM)).copy()
    out["b2rep"] = np.broadcast_to(
        inp["b2"].astype(np.float32), (P, WDIM)).copy()

    # x augmented with ones column, padded to PAD_N, per-core rolled
    x = inp["x"].astype(np.float32)
    xa = np.zeros((PAD_N, KA), np.float32)
    xa[:N, :DIN] = x
    xa[:N, DIN] = 1.0
    nab = PAD_N // (ABATCH * P)
    xaT, xaTown = [], []
    for c in range(NCORES):
        xr = np.roll(xa, -c * NPC, axis=0)
        xaT.append(np.ascontiguousarray(
            xr.reshape(nab, ABATCH * P, KA).transpose(0, 2, 1)).astype(BF16))
        xaTown.append(np.ascontiguousarray(
            xr[:NPC].reshape(TILES_OWN, P, KA).transpose(0, 2, 1)
        ).astype(BF16))
    out["xaT"] = xaT
    out["xaTown"] = xaTown

    out["iota"] = np.broadcast_to(
        np.arange(P, dtype=np.float32), (P, P)).astype(BF16).copy()
    out["iotac"] = np.arange(P, dtype=np.float32)[:, None].astype(BF16)
    out["ident"] = np.eye(P, dtype=np.float32).astype(BF16)
    out["ones"] = np.ones((1, P), np.float32).astype(BF16)
    return out


# ---------------------------------------------------------------------------
# Host-side edge planning
# ---------------------------------------------------------------------------

def _plan_edges(edge_index):
    src = edge_index[0].astype(np.int64)
    dst = edge_index[1].astype(np.int64)
    loops = np.arange(N, dtype=np.int64)
    src = np.concatenate([src, loops])
    dst = np.concatenate([dst, loops])
    core_of = dst // NPC

    per = [[None] * TILES_OWN for _ in range(NCORES)]
    nlo = np.zeros((NCORES, TILES_OWN), np.int64)
    nhi = np.zeros((NCORES, TILES_OWN), np.int64)
    for c in range(NCORES):
        m = core_of == c
        # rotate into the core's numbering: own dst -> [0, NPC)
        s = (src[m] - c * NPC) % PAD_N
        dl = dst[m] - c * NPC
        t = dl // P
        islo = s < SPLIT
        for ti in range(TILES_OWN):
            mt = t == ti
            per[c][ti] = (s[mt & islo], dl[mt & islo] % P,
                          s[mt & ~islo] - SPLIT, dl[mt & ~islo] % P)
            nlo[c, ti] = int((mt & islo).sum())
            nhi[c, ti] = int((mt & ~islo).sum())

    clo = np.maximum(_ceil(nlo.max(0), P), 1)
    chi = np.maximum(_ceil(nhi.max(0), P), 1)
    C = clo + chi
    coloff = np.concatenate([[0], np.cumsum(C)])
    totc = int(coloff[-1])

    idx16 = np.zeros((NCORES, 16, totc * 8), np.int16)
    dstloc = np.full((NCORES, P, totc), 255.0, np.float32)
    for c in range(NCORES):
        for ti in range(TILES_OWN):
            slo, dlo, shi, dhi = per[c][ti]
            base = int(coloff[ti])
            for (ss, dd, off, nch) in (
                (slo, dlo, base, int(clo[ti])),
                (shi, dhi, base + int(clo[ti]), int(chi[ti])),
            ):
                n = nch * P
                flat = np.zeros(n, np.int16)
                flat[:len(ss)] = ss.astype(np.int16)
                idx16[c, :, off * 8:off * 8 + n // 16] = \
                    flat.reshape(n // 16, 16).T
                r = np.arange(len(dd))
                dstloc[c, r % P, off + r // P] = dd
    idx16 = np.tile(idx16, (1, 8, 1))  # replicate for the 8 gpsimd cores
    return {
        "C": tuple(int(v) for v in C),
        "clo": tuple(int(v) for v in clo),
        "totc": totc,
        "idx16": np.ascontiguousarray(idx16),
        "dstloc": dstloc.astype(BF16),
    }


# ---------------------------------------------------------------------------
# Device program builders
# ---------------------------------------------------------------------------

class _QRR:
    def __init__(self, nq):
        self.i, self.nq = 0, nq

    def __call__(self):
        q = self.i % self.nq
        self.i += 1
        return q


def _emit_edge_phase(nc, pools, C, clo, coloff, glo_ap, ghi_ap, sd_ap,
                     idx_sb, loc_sb, iota_sb, iotac_sb, ident_sb, cmax, qrr,
                     tile_epilogue):
    """Edge aggregation over destination tiles. PSUM accumulator layout:
    cols 0:WDIM = sum(ex*h), cols WDIM:GW = sum(ex) per head."""
    sbp, psB, sde_p, psLT = pools
    for t in range(len(C)):
        ct, cl = C[t], clo[t]
        base = int(coloff[t])
        g = sbp.tile([P, cmax, GP], BF, tag="gather")
        for (c0, c1, table) in ((0, cl, glo_ap), (cl, ct, ghi_ap)):
            for s in range(c0, c1, GMAX):
                e = min(s + GMAX, c1)
                nc.gpsimd.dma_gather(
                    out_ap=g[:, s:e, :], in_ap=table,
                    idxs_ap=idx_sb[:, (base + s) * 8:(base + e) * 8],
                    num_idxs=(e - s) * P, num_idxs_reg=(e - s) * P,
                    elem_size=GP, queue_num=qrr())
        sdt = sbp.tile([P, HEADS], BF, tag="sdt")
        nc.sync.dma_start(sdt[:], sd_ap[t * P:(t + 1) * P, :])

        # transposed one-hot [m, e] = (m == dstloc[e]), groups of 4 chunks
        oht = sbp.tile([P, cmax, P], BF, tag="oht")
        for s in range(0, ct, 4):
            e = min(s + 4, ct)
            pslt = psLT.tile([P, 4, P], BF, space="PSUM", tag="psLT")
            for c in range(s, e):
                nc.tensor.transpose(
                    out=pslt[:, c - s, :],
                    in_=loc_sb[:, base + c:base + c + 1].to_broadcast([P, P]),
                    identity=ident_sb[:])
            nc.vector.tensor_tensor(
                out=oht[:, s:e, :],
                in0=iotac_sb[:].unsqueeze(-1).to_broadcast([P, e - s, P]),
                in1=pslt[:, 0:e - s, :],
                op=mybir.AluOpType.is_equal)
        sde = sde_p.tile([P, cmax * HEADS], F32, space="PSUM", tag="sde")
        for c in range(ct):
            nc.tensor.matmul(out=sde[:, c * HEADS:(c + 1) * HEADS],
                             lhsT=oht[:, c, :], rhs=sdt[:],
                             start=True, stop=True)

        score = sbp.tile([P, cmax * HEADS], F32, tag="score")
        nc.vector.tensor_tensor(
            out=score[:, 0:ct * HEADS].rearrange("p (c h) -> p c h", h=HEADS),
            in0=g[:, 0:ct, WDIM:GW],
            in1=sde[:, 0:ct * HEADS].rearrange("p (c h) -> p c h", h=HEADS),
            op=mybir.AluOpType.add)
        score2 = sbp.tile([P, cmax * HEADS], F32, tag="score2")
        nc.scalar.activation(score2[:, 0:ct * HEADS], score[:, 0:ct * HEADS],
                             mybir.ActivationFunctionType.Lrelu,
                             alpha=NEG_SLOPE)
        nc.scalar.activation(
            g[:, 0:ct, WDIM:GW],
            score2[:, 0:ct * HEADS].rearrange("p (c h) -> p c h", h=HEADS),
            mybir.ActivationFunctionType.Exp)
        nc.vector.tensor_tensor(
            out=g[:, 0:ct, 0:WDIM].rearrange("p c (h d) -> p c h d", h=HEADS),
            in0=g[:, 0:ct, 0:WDIM].rearrange("p c (h d) -> p c h d", h=HEADS),
            in1=g[:, 0:ct, WDIM:GW].unsqueeze(-1).to_broadcast(
                [P, ct, HEADS, HID]),
            op=mybir.AluOpType.mult)

        oh = sbp.tile([P, cmax, P], BF, tag="ohagg")
        nc.vector.tensor_tensor(
            out=oh[:, 0:ct, :],
            in0=loc_sb[:, base:base + ct].unsqueeze(-1).to_broadcast(
                [P, ct, P]),
            in1=iota_sb[:].unsqueeze(1).to_broadcast([P, ct, P]),
            op=mybir.AluOpType.is_equal)
        psumB = psB.tile([P, GW], F32, space="PSUM", tag="psumB")
        for c in range(ct):
            nc.tensor.matmul(out=psumB[:], lhsT=oh[:, c, :],
                             rhs=g[:, c, 0:GW],
                             start=(c == 0), stop=(c == ct - 1))
        tile_epilogue(t, psumB)


def _emit_softmax_elu(nc, sbp, psumB, brep_sb):
    den = sbp.tile([P, HEADS], F32, tag="den")
    nc.vector.tensor_scalar(
        out=den[:], in0=psumB[:, WDIM:GW], scalar1=1e-30, scalar2=None,
        op0=mybir.AluOpType.max)
    recip = sbp.tile([P, HEADS], F32, tag="recip")
    nc.vector.reciprocal(recip[:], den[:])
    ob = sbp.tile([P, WDIM], F32, tag="aggb")
    nc.vector.tensor_tensor(
        out=ob[:].rearrange("p (h d) -> p h d", h=HEADS),
        in0=psumB[:, 0:WDIM].rearrange("p (h d) -> p h d", h=HEADS),
        in1=recip[:].unsqueeze(-1).to_broadcast([P, HEADS, HID]),
        op=mybir.AluOpType.mult)
    nc.vector.tensor_tensor(
        out=ob[:], in0=ob[:], in1=brep_sb[:], op=mybir.AluOpType.add)
    neg = sbp.tile([P, WDIM], F32, tag="neg")
    nc.vector.tensor_scalar(
        out=neg[:], in0=ob[:], scalar1=0.0, scalar2=None,
        op0=mybir.AluOpType.min)
    en = sbp.tile([P, WDIM], F32, tag="en")
    nc.scalar.activation(en[:], neg[:], mybir.ActivationFunctionType.Exp)
    pm1 = sbp.tile([P, WDIM], F32, tag="pm1")
    nc.vector.tensor_scalar(
        out=pm1[:], in0=ob[:], scalar1=0.0, scalar2=-1.0,
        op0=mybir.AluOpType.max, op1=mybir.AluOpType.add)
    e = sbp.tile([P, WDIM], F32, tag="e")
    nc.vector.tensor_tensor(
        out=e[:], in0=en[:], in1=pm1[:], op=mybir.AluOpType.add)
    return e


def _emit_transpose_halves(nc, sbp, psp, e, ident_sb):
    eb = sbp.tile([P, WDIM], BF, tag="eb")
    nc.vector.tensor_copy(out=eb[:], in_=e[:])
    eTs = []
    for half in range(2):
        pst = psp.tile([P, P], BF, space="PSUM", tag="psT")
        nc.tensor.transpose(
            out=pst[:], in_=eb[:, half * P:(half + 1) * P],
            identity=ident_sb[:])
        eT = sbp.tile([P, P], BF, tag=f"eT{half}")
        nc.vector.tensor_copy(out=eT[:], in_=pst[:])
        eTs.append(eT)
    return eTs


def _mk_bass():
    return bacc.Bacc("TRN2", target_bir_lowering=False, debug=False,
                     enable_asserts=False, num_devices=NCORES,
                     num_swdge_queues=4)


def _build_launch1(C, clo, totc):
    nab = PAD_N // (ABATCH * P)
    coloff = np.concatenate([[0], np.cumsum(C)])
    cmax = int(max(C))
    nc = _mk_bass()
    dt = nc.dram_tensor
    xaT = dt("xaT", [nab, KA, ABATCH * P], BF, kind="ExternalInput").ap()
    W1aug = dt("W1aug", [KA, AW], BF, kind="ExternalInput").ap()
    W2a0 = dt("W2a0", [P, AW], BF, kind="ExternalInput").ap()
    W2a1 = dt("W2a1", [P, AW], BF, kind="ExternalInput").ap()
    W2d = dt("W2d", [1, AW], BF, kind="ExternalInput").ap()
    iota = dt("iota", [P, P], BF, kind="ExternalInput").ap()
    iotac = dt("iotac", [P, 1], BF, kind="ExternalInput").ap()
    ident = dt("ident", [P, P], BF, kind="ExternalInput").ap()
    ones = dt("ones", [1, P], BF, kind="ExternalInput").ap()
    b1rep = dt("b1rep", [P, WDIM], F32, kind="ExternalInput").ap()
    idx16 = dt("idx16", [P, totc * 8], I16, kind="ExternalInput").ap()
    dstloc = dt("dstloc", [P, totc], BF, kind="ExternalInput").ap()
    g2own = dt("g2own", [NPC, GW], BF, kind="ExternalOutput").ap()
    sd2own = dt("sd2own", [NPC, HEADS], BF, kind="ExternalOutput").ap()
    G1 = dt("G1", [PAD_N, GP], BF).ap()
    SD1 = dt("SD1", [PAD_N, HEADS], BF).ap()

    qrr = _QRR(4)
    with tile.TileContext(nc) as tc:
        with (
            tc.tile_pool(name="consts", bufs=1) as cst,
            tc.tile_pool(name="sbuf", bufs=3) as sbp,
            tc.tile_pool(name="sbA", bufs=2) as sbA,
            tc.tile_pool(name="sb2", bufs=2) as sb2,
            tc.tile_pool(name="psA", bufs=2, space="PSUM") as psA,
            tc.tile_pool(name="psB", bufs=2, space="PSUM") as psB,
            tc.tile_pool(name="sde", bufs=1, space="PSUM") as sde_p,
            tc.tile_pool(name="psLT", bufs=1, space="PSUM") as psLT,
            tc.tile_pool(name="psT", bufs=2, space="PSUM") as psT,
        ):
            def cload(ap, shape, dtype):
                t = cst.tile(shape, dtype, tag=ap.tensor.name)
                nc.sync.dma_start(t[:], ap[:])
                return t

            W1aug_sb = cload(W1aug, [KA, AW], BF)
            W2a0_sb = cload(W2a0, [P, AW], BF)
            W2a1_sb = cload(W2a1, [P, AW], BF)
            W2d_sb = cload(W2d, [1, AW], BF)
            iota_sb = cload(iota, [P, P], BF)
            iotac_sb = cload(iotac, [P, 1], BF)
            ident_sb = cload(ident, [P, P], BF)
            ones_sb = cload(ones, [1, P], BF)
            b1rep_sb = cload(b1rep, [P, WDIM], F32)
            idx_sb = cload(idx16, [P, totc * 8], I16)
            loc_sb = cload(dstloc, [P, totc], BF)

            # ---- phase A: full-graph node transform (replicated) ----
            for it in range(nab):
                xt = sbA.tile([KA, ABATCH * P], BF, tag="xt")
                nc.sync.dma_start(xt[:], xaT[it])
                gsb = sbA.tile([P, ABATCH, GW], BF, tag="gsb")
                sdsb = sbA.tile([P, ABATCH, HEADS], BF, tag="sdsb")
                for b in range(ABATCH):
                    psa = psA.tile([P, AW], F32, space="PSUM", tag="psumA")
                    nc.tensor.matmul(out=psa[:],
                                     lhsT=xt[:, b * P:(b + 1) * P],
                                     rhs=W1aug_sb[:], start=True, stop=True)
                    nc.vector.tensor_copy(out=gsb[:, b, :], in_=psa[:, 0:GW])
                    nc.vector.tensor_copy(out=sdsb[:, b, :],
                                          in_=psa[:, GW:AW])
                r0 = it * ABATCH * P
                nc.sync.dma_start(
                    G1[r0:r0 + ABATCH * P, 0:GW].rearrange(
                        "(b p) d -> p b d", p=P), gsb[:])
                nc.sync.dma_start(
                    SD1[r0:r0 + ABATCH * P, :].rearrange(
                        "(b p) d -> p b d", p=P), sdsb[:])

            tc.strict_bb_all_engine_barrier()

            # ---- conv1 edge phase + conv2 node transform ----
            def epilogue(t, psumB):
                e1 = _emit_softmax_elu(nc, sb2, psumB, b1rep_sb)
                eTs = _emit_transpose_halves(nc, sb2, psT, e1, ident_sb)
                psa2 = psA.tile([P, AW], F32, space="PSUM", tag="psumA")
                nc.tensor.matmul(out=psa2[:], lhsT=ones_sb[:], rhs=W2d_sb[:],
                                 start=True, stop=False)
                nc.tensor.matmul(out=psa2[:], lhsT=eTs[0][:], rhs=W2a0_sb[:],
                                 start=False, stop=False)
                nc.tensor.matmul(out=psa2[:], lhsT=eTs[1][:], rhs=W2a1_sb[:],
                                 start=False, stop=True)
                g2 = sb2.tile([P, GW], BF, tag="g2")
                nc.vector.tensor_copy(out=g2[:], in_=psa2[:, 0:GW])
                nc.sync.dma_start(g2own[t * P:(t + 1) * P, :], g2[:])
                sd2 = sb2.tile([P, HEADS], BF, tag="sd2")
                nc.vector.tensor_copy(out=sd2[:], in_=psa2[:, GW:AW])
                nc.sync.dma_start(sd2own[t * P:(t + 1) * P, :], sd2[:])

            _emit_edge_phase(nc, (sbp, psB, sde_p, psLT), C, clo, coloff,
                             G1[:], G1[SPLIT:, :], SD1, idx_sb, loc_sb,
                             iota_sb, iotac_sb, ident_sb, cmax, qrr, epilogue)
    nc.compile()
    return nc


def _build_launch2(C, clo, totc):
    coloff = np.concatenate([[0], np.cumsum(C)])
    cmax = int(max(C))
    nc = _mk_bass()
    dt = nc.dram_tensor
    G2 = dt("G2", [PAD_N, GP], BF, kind="ExternalInput").ap()
    SD2 = dt("SD2", [PAD_N, HEADS], BF, kind="ExternalInput").ap()
    iota = dt("iota", [P, P], BF, kind="ExternalInput").ap()
    iotac = dt("iotac", [P, 1], BF, kind="ExternalInput").ap()
    ident = dt("ident", [P, P], BF, kind="ExternalInput").ap()
    b2rep = dt("b2rep", [P, WDIM], F32, kind="ExternalInput").ap()
    P1a0 = dt("P1a0", [P, HID], BF, kind="ExternalInput").ap()
    P1a1 = dt("P1a1", [P, HID], BF, kind="ExternalInput").ap()
    P1baug = dt("P1baug", [KA, HID], BF, kind="ExternalInput").ap()
    p2 = dt("p2", [HID, 1], BF, kind="ExternalInput").ap()
    p2brep = dt("p2brep", [P, 1], F32, kind="ExternalInput").ap()
    xaTown = dt("xaTown", [TILES_OWN, KA, P], BF, kind="ExternalInput").ap()
    idx16 = dt("idx16", [P, totc * 8], I16, kind="ExternalInput").ap()
    dstloc = dt("dstloc", [P, totc], BF, kind="ExternalInput").ap()
    y = dt("y", [NPC, 1], F32, kind="ExternalOutput").ap()

    qrr = _QRR(4)
    with tile.TileContext(nc) as tc:
        with (
            tc.tile_pool(name="consts", bufs=1) as cst,
            tc.tile_pool(name="sbuf", bufs=3) as sbp,
            tc.tile_pool(name="sb2", bufs=2) as sb2,
            tc.tile_pool(name="psB", bufs=2, space="PSUM") as psB,
            tc.tile_pool(name="sde", bufs=1, space="PSUM") as sde_p,
            tc.tile_pool(name="psLT", bufs=1, space="PSUM") as psLT,
            tc.tile_pool(name="psT", bufs=1, space="PSUM") as psT,
            tc.tile_pool(name="psC", bufs=1, space="PSUM") as psC,
            tc.tile_pool(name="psT2", bufs=1, space="PSUM") as psT2,
            tc.tile_pool(name="psY", bufs=1, space="PSUM") as psY,
        ):
            def cload(ap, shape, dtype):
                t = cst.tile(shape, dtype, tag=ap.tensor.name)
                nc.sync.dma_start(t[:], ap[:])
                return t

            iota_sb = cload(iota, [P, P], BF)
            iotac_sb = cload(iotac, [P, 1], BF)
            ident_sb = cload(ident, [P, P], BF)
            b2rep_sb = cload(b2rep, [P, WDIM], F32)
            P1a0_sb = cload(P1a0, [P, HID], BF)
            P1a1_sb = cload(P1a1, [P, HID], BF)
            P1baug_sb = cload(P1baug, [KA, HID], BF)
            p2_sb = cload(p2, [HID, 1], BF)
            p2b_sb = cload(p2brep, [P, 1], F32)
            idx_sb = cload(idx16, [P, totc * 8], I16)
            loc_sb = cload(dstloc, [P, totc], BF)

            def epilogue(t, psumB):
                e2 = _emit_softmax_elu(nc, sb2, psumB, b2rep_sb)
                eTs = _emit_transpose_halves(nc, sb2, psT, e2, ident_sb)
                xt = sb2.tile([KA, P], BF, tag="xt")
                nc.sync.dma_start(xt[:], xaTown[t])
                psc = psC.tile([P, HID], F32, space="PSUM", tag="psumC")
                nc.tensor.matmul(out=psc[:], lhsT=eTs[0][:], rhs=P1a0_sb[:],
                                 start=True, stop=False)
                nc.tensor.matmul(out=psc[:], lhsT=eTs[1][:], rhs=P1a1_sb[:],
                                 start=False, stop=False)
                nc.tensor.matmul(out=psc[:], lhsT=xt[:], rhs=P1baug_sb[:],
                                 start=False, stop=True)
                tt = sb2.tile([P, HID], BF, tag="tt")
                nc.scalar.activation(tt[:], psc[:],
                                     mybir.ActivationFunctionType.Relu)
                pst2 = psT2.tile([HID, P], BF, space="PSUM", tag="psumT2")
                nc.tensor.transpose(out=pst2[:], in_=tt[:],
                                    identity=ident_sb[:])
                ttT = sb2.tile([HID, P], BF, tag="ttT")
                nc.vector.tensor_copy(out=ttT[:], in_=pst2[:])
                psy = psY.tile([P, 1], F32, space="PSUM", tag="psumY")
                nc.tensor.matmul(out=psy[:], lhsT=ttT[:], rhs=p2_sb[:],
                                 start=True, stop=True)
                ysb = sb2.tile([P, 1], F32, tag="ysb")
                nc.scalar.activation(ysb[:], psy[:],
                                     mybir.ActivationFunctionType.Identity,
                                     bias=p2b_sb[:])
                nc.sync.dma_start(y[t * P:(t + 1) * P, :], ysb[:])

            _emit_edge_phase(nc, (sbp, psB, sde_p, psLT), C, clo, coloff,
                             G2, G2[SPLIT:, :], SD2, idx_sb, loc_sb,
                             iota_sb, iotac_sb, ident_sb, cmax, qrr, epilogue)
    nc.compile()
    return nc


# ---------------------------------------------------------------------------
# Entry point
# ---------------------------------------------------------------------------

def _get_programs(C, clo, totc):
    key = (C, clo, totc)
    if key not in _PROG_CACHE:
        _PROG_CACHE[key] = (_build_launch1(C, clo, totc),
                            _build_launch2(C, clo, totc))
    return _PROG_CACHE[key]


def kernel(**inputs):
    cfg = _fold(inputs)
    plan = _plan_edges(np.asarray(inputs["edge_index"]))
    C, clo, totc = plan["C"], plan["clo"], plan["totc"]
    nc1, nc2 = _get_programs(C, clo, totc)

    shared1 = {k: cfg[k] for k in ["W1aug", "W2a0", "W2a1", "W2d", "iota",
                                   "iotac", "ident", "ones", "b1rep"]}
    in_maps1 = []
    for c in range(NCORES):
        m = dict(shared1)
        m["xaT"] = cfg["xaT"][c]
        m["idx16"] = plan["idx16"][c]
        m["dstloc"] = plan["dstloc"][c]
        in_maps1.append(m)
    res1 = run_bass_kernel_spmd(nc1, in_maps1, list(range(NCORES)),
                                trace=TRACE, **TRACE_KW)

    G2 = np.zeros((PAD_N, GP), BF16)
    SD2 = np.zeros((PAD_N, HEADS), BF16)
    for c in range(NCORES):
        nr = _npc_real(c)
        G2[c * NPC:c * NPC + nr, 0:GW] = res1.results[c]["g2own"][:nr]
        SD2[c * NPC:c * NPC + nr] = res1.results[c]["sd2own"][:nr]

    shared2 = {k: cfg[k] for k in ["iota", "iotac", "ident", "b2rep", "P1a0",
                                   "P1a1", "P1baug", "p2", "p2brep"]}
    in_maps2 = []
    for c in range(NCORES):
        m = dict(shared2)
        m["G2"] = np.roll(G2, -c * NPC, axis=0)
        m["SD2"] = np.roll(SD2, -c * NPC, axis=0)
        m["xaTown"] = cfg["xaTown"][c]
        m["idx16"] = plan["idx16"][c]
        m["dstloc"] = plan["dstloc"][c]
        in_maps2.append(m)
    res2 = run_bass_kernel_spmd(nc2, in_maps2, list(range(NCORES)),
                                trace=TRACE, **TRACE_KW)
    y = np.concatenate([res2.results[c]["y"][:_npc_real(c)]
                        for c in range(NCORES)], 0)
    kernel.last_exec_ns = (
        (res1.exec_time_ns or 0) + (res2.exec_time_ns or 0)) or None
    kernel.last_results = (res1, res2)
    return y.astype(np.float32)
